# revision 1
# baseline (speedup 1.0000x reference)
"""Trainium2 Bass kernel for FastMaskedDense1D.update_site (index=300 regime).

Math (reference semantics, EXCLUSIVE=1):
    cache[:, index-1, :] = inputs                      (scatter)
    cache_i = cache[:, :index+1, :].reshape(B, -1)
    kernel_i = kernel.reshape(S, IF, S, F)[:index+1, :, index, :]
    kernel_i *= (arange(index+1) <= index-1)[:, None, None]   (mask)
    y = cache_i @ kernel_i.reshape(-1, F) + bias[index]

Because the mask zeroes site `index`, only sites 0..index-1 contribute, with
site index-1 replaced by `inputs`. So the whole op is one skinny matmul:
    y = A @ Keff + bias,  A: (B, index*IF), Keff: (index*IF, F)

Strategy: data-parallel over the batch across 8 NeuronCores. The host folds
scatter + mask + kernel-slice + bias (ones-column trick) into a per-core
dense problem laid out as A^T (contraction-major, contiguous) so the device
does a single streaming matmul  out^T = Keff^T @ A^T  at full DMA rate:
  - AT  (K_pad, 1024) f32 per core  — the big streamed operand (~20 MB)
  - KM  (128, T*F) f32 replicated   — Keff swizzled so tile t's stationary
                                      block is kw[:, t*F:(t+1)*F]
  - out (F, 1024) f32 per core      — transposed back on the host
"""

import math

import numpy as np

BATCH = 8192
SIZE = 512
FEATURES = 16
IN_FEATURES = 16
EXCLUSIVE = 1
NCORES = 8
P = 128
G = 4  # contraction tiles per DMA (2 MB transfers)

_NC_CACHE: dict = {}


def _build(K_pad: int, B: int, F: int, repeats: int = 1):
    """out(F, B) = KM_unswizzled(K_pad, F).T @ AT(K_pad, B)."""
    import concourse.bacc as bacc
    import concourse.mybir as mybir
    from concourse.tile import TileContext

    F32 = mybir.dt.float32
    T = K_pad // P
    assert B % 512 == 0
    NBH = B // 512

    nc = bacc.Bacc("TRN2", target_bir_lowering=False, debug=False)
    AT = nc.dram_tensor("at", (K_pad, B), F32, kind="ExternalInput")
    KM = nc.dram_tensor("km", (P, T * F), F32, kind="ExternalInput")
    OUT = nc.dram_tensor("out", (F, B), F32, kind="ExternalOutput")

    at_view = AT.ap().rearrange("(t p) b -> p t b", p=P)

    with TileContext(nc) as tc:
        with (
            tc.tile_pool(name="kw", bufs=1) as kwpool,
            tc.tile_pool(name="a", bufs=3) as apool,
            tc.tile_pool(name="o", bufs=2) as opool,
            tc.tile_pool(name="ps", bufs=2 * NBH, space="PSUM") as pspool,
        ):
            kw = kwpool.tile([P, T * F], F32)
            nc.sync.dma_start(kw[:], KM.ap())
            for _ in range(repeats):
                psums = [
                    pspool.tile([F, 512], F32, tag=f"ps{bh}", name=f"psum{bh}")
                    for bh in range(NBH)
                ]
                t = 0
                while t < T:
                    g = min(G, T - t)
                    a_tile = apool.tile([P, G, B], F32, tag="a")
                    nc.sync.dma_start(a_tile[:, :g, :], at_view[:, t : t + g, :])
                    for gi in range(g):
                        tt = t + gi
                        for bh in range(NBH):
                            nc.tensor.matmul(
                                psums[bh][:],
                                kw[:, tt * F : (tt + 1) * F],
                                a_tile[:, gi, bh * 512 : (bh + 1) * 512],
                                start=(tt == 0),
                                stop=(tt == T - 1),
                            )
                    t += g
                outsb = opool.tile([F, B], F32, tag="out")
                for bh in range(NBH):
                    nc.any.tensor_copy(
                        out=outsb[:, bh * 512 : (bh + 1) * 512], in_=psums[bh][:]
                    )
                nc.sync.dma_start(OUT.ap(), outsb[:])
    nc.compile()
    return nc


def _get_nc(K_pad: int, B: int, F: int, repeats: int = 1):
    key = (K_pad, B, F, repeats)
    if key not in _NC_CACHE:
        _NC_CACHE[key] = _build(K_pad, B, F, repeats)
    return _NC_CACHE[key]


def _prepare(inputs, cache, kernel, bias, index):
    """Host-side fold: returns (in_maps, K_pad, B_core, F)."""
    index = int(index)
    B, IF = inputs.shape
    S, F = bias.shape
    assert B % NCORES == 0
    B_core = B // NCORES

    hi = index - EXCLUSIVE
    n_sites = hi + 1 if hi >= 0 else 0  # contributing cache sites 0..n_sites-1
    K_len = n_sites * IF + 1  # +1 = ones column carrying the bias
    K_pad = max(P, math.ceil(K_len / P) * P)

    # Keff (masked kernel slice) + bias row, zero-padded, then swizzled so
    # KM[p, t*F + n] = Keff_pad[t*128 + p, n].
    km = np.zeros((K_pad, F), np.float32)
    if n_sites:
        kr = kernel.reshape(S, IF, S, F)[:n_sites, :, index, :]
        km[: n_sites * IF] = np.asarray(kr, np.float32).reshape(n_sites * IF, F)
    km[n_sites * IF] = np.asarray(bias[index], np.float32)
    T = K_pad // P
    KM = np.ascontiguousarray(
        km.reshape(T, P, F).transpose(1, 0, 2).reshape(P, T * F)
    )

    inputs = np.asarray(inputs, np.float32)
    cache = np.asarray(cache, np.float32)
    in_maps = []
    for c in range(NCORES):
        rows = slice(c * B_core, (c + 1) * B_core)
        at = np.zeros((K_pad, B_core), np.float32)
        if n_sites:
            at[: n_sites * IF] = (
                cache[rows, :n_sites, :].reshape(B_core, n_sites * IF).T
            )
            at[hi * IF : (hi + 1) * IF] = inputs[rows].T
        at[n_sites * IF] = 1.0
        in_maps.append({"at": at, "km": KM})
    return in_maps, K_pad, B_core, F


def kernel(inputs, cache, kernel, bias, index):
    from concourse.bass_utils import run_bass_kernel_spmd

    in_maps, K_pad, B_core, F = _prepare(inputs, cache, kernel, bias, index)
    nc = _get_nc(K_pad, B_core, F)
    res = run_bass_kernel_spmd(nc, in_maps, core_ids=list(range(NCORES)))
    out = np.concatenate(
        [np.asarray(res.results[c]["out"]).T for c in range(NCORES)], axis=0
    )
    return np.ascontiguousarray(out, dtype=np.float32)


def bench(inputs, cache, kernel, bias, index, repeats=33, iters=3):
    """Estimate per-invocation HW exec time by amortizing an in-NEFF repeat
    loop: t(repeats) - t(1) cancels upload/dispatch overhead."""
    import time

    from concourse.bass_utils import run_bass_kernel_spmd

    in_maps, K_pad, B_core, F = _prepare(inputs, cache, kernel, bias, index)

    def timed(rep):
        nc = _get_nc(K_pad, B_core, F, rep)
        ts = []
        for _ in range(iters):
            t0 = time.perf_counter()
            run_bass_kernel_spmd(nc, in_maps, core_ids=list(range(NCORES)))
            ts.append(time.perf_counter() - t0)
        return min(ts)

    t1 = timed(1)
    tR = timed(repeats)
    return (tR - t1) / (repeats - 1), t1, tR


# revision 2
# speedup vs baseline: 83.6106x; 83.6106x over previous
"""Trainium2 Bass kernel for FastMaskedDense1D.update_site (index=300 regime).

Math (reference semantics, EXCLUSIVE=1):
    cache[:, index-1, :] = inputs                      (scatter)
    cache_i = cache[:, :index+1, :].reshape(B, -1)
    kernel_i = kernel.reshape(S, IF, S, F)[:index+1, :, index, :]
    kernel_i *= (arange(index+1) <= index-1)[:, None, None]   (mask)
    y = cache_i @ kernel_i.reshape(-1, F) + bias[index]

Because the mask zeroes site `index`, only sites 0..index-1 contribute, with
site index-1 replaced by `inputs`. So the whole op is one skinny matmul:
    y = A @ Keff + bias,  A: (B, index*IF), Keff: (index*IF, F)

Strategy: data-parallel over the batch across 8 NeuronCores. The host folds
scatter + mask + kernel-slice + bias (ones-column trick) into a per-core
dense problem laid out contraction-major (A^T, contiguous) so each core runs
a single streaming matmul  out^T = Keff^T @ A^T  at full DMA rate.

The PE's native fp32 matmul runs at 4 cycles/row, which would make the
TensorEngine the bottleneck (~65us/core vs ~55us DMA). Instead the host
splits fp32 into a bf16 hi/lo pair (hi = bf16(x), lo = bf16(x - hi)) and the
device computes 3 full-rate bf16 matmuls accumulated in fp32 PSUM:
    A@K ~= Ahi@Khi + Alo@Khi + Ahi@Klo     (lo@lo dropped, ~4e-6 relative)
Same DRAM traffic as fp32 (2 x bf16), ~3e-6 relative error, PE off the
critical path (~50us < DMA ~55us).

DRAM layout per core:
  at2 (K_pad*2, 1024) bf16 : k-tile-interleaved rows
                             [t0_hi(128) | t0_lo(128) | t1_hi(128) | ...]
  km2 (128, T*2*F) bf16    : km2[p, (t*2+h)*F + n] = Ksplit[t*128+p, h, n]
  out (F, 1024) f32        : out^T; host transposes back
"""

import math

import numpy as np

BATCH = 8192
SIZE = 512
FEATURES = 16
IN_FEATURES = 16
EXCLUSIVE = 1
NCORES = 8
P = 128
G = 4  # contraction k-tiles per DMA (1 MB transfers with split_b=2)
SPLIT_B = 2  # DMAs per supertile along the batch dim (finer dep granularity)
BUFS = 3

_NC_CACHE: dict = {}


def _build(K_pad: int, B: int, F: int, repeats: int = 1):
    """out(F, B) = sum_t Ksplit[t].T @ Asplit[t] over bf16 hi/lo pairs."""
    import concourse.bacc as bacc
    import concourse.mybir as mybir
    from concourse.tile import TileContext

    F32 = mybir.dt.float32
    BF16 = mybir.dt.bfloat16
    T = K_pad // P
    assert B % 512 == 0
    NBH = B // 512

    nc = bacc.Bacc("TRN2", target_bir_lowering=False, debug=False)
    AT2 = nc.dram_tensor("at2", (K_pad * 2, B), BF16, kind="ExternalInput")
    KM2 = nc.dram_tensor("km2", (P, T * 2 * F), BF16, kind="ExternalInput")
    OUT = nc.dram_tensor("out", (F, B), F32, kind="ExternalOutput")

    # q = t*2 + h  (k-tile t, half h: 0=hi, 1=lo)
    at_view = AT2.ap().rearrange("(q p) b -> p q b", p=P)

    with TileContext(nc) as tc:
        with (
            tc.tile_pool(name="kw", bufs=1) as kwpool,
            tc.tile_pool(name="a", bufs=BUFS) as apool,
            tc.tile_pool(name="o", bufs=2) as opool,
            tc.tile_pool(name="ps", bufs=2 * NBH, space="PSUM") as pspool,
        ):
            kw = kwpool.tile([P, T * 2 * F], BF16)
            nc.sync.dma_start(kw[:], KM2.ap())

            def kwt(t, h):
                o = (t * 2 + h) * F
                return kw[:, o : o + F]

            for _ in range(repeats):
                psums = [
                    pspool.tile([F, 512], F32, tag=f"ps{bh}", name=f"psum{bh}")
                    for bh in range(NBH)
                ]
                t = 0
                while t < T:
                    g = min(G, T - t)
                    a_tile = apool.tile([P, G * 2, B], BF16, tag="a", name="a_tile")
                    bs = B // SPLIT_B
                    for sb in range(SPLIT_B):
                        nc.sync.dma_start(
                            a_tile[:, : g * 2, sb * bs : (sb + 1) * bs],
                            at_view[:, t * 2 : (t + g) * 2, sb * bs : (sb + 1) * bs],
                        )
                    for gi in range(g):
                        tt = t + gi
                        for bh in range(NBH):
                            bsl = slice(bh * 512, (bh + 1) * 512)
                            # (K half, A half): (hi,hi), (lo,hi), (hi,lo)
                            for j, (kh, ah) in enumerate(((0, 0), (1, 0), (0, 1))):
                                nc.tensor.matmul(
                                    psums[bh][:],
                                    kwt(tt, kh),
                                    a_tile[:, gi * 2 + ah, bsl],
                                    start=(tt == 0 and j == 0),
                                    stop=(tt == T - 1 and j == 2),
                                )
                    t += g
                outsb = opool.tile([F, B], F32, tag="out", name="outsb")
                for bh in range(NBH):
                    nc.any.tensor_copy(
                        out=outsb[:, bh * 512 : (bh + 1) * 512], in_=psums[bh][:]
                    )
                nc.sync.dma_start(OUT.ap(), outsb[:])
    nc.compile()
    return nc


def _get_nc(K_pad: int, B: int, F: int, repeats: int = 1):
    key = (K_pad, B, F, repeats)
    if key not in _NC_CACHE:
        _NC_CACHE[key] = _build(K_pad, B, F, repeats)
    return _NC_CACHE[key]


def _split_bf16(x):
    import ml_dtypes

    hi = x.astype(ml_dtypes.bfloat16)
    lo = (x - hi.astype(np.float32)).astype(ml_dtypes.bfloat16)
    return hi, lo


def _prepare(inputs, cache, kernel, bias, index):
    """Host-side fold: returns (in_maps, K_pad, B_core, F)."""
    index = int(index)
    B, IF = inputs.shape
    S, F = bias.shape
    assert B % NCORES == 0
    B_core = B // NCORES

    hi_site = index - EXCLUSIVE
    n_sites = hi_site + 1 if hi_site >= 0 else 0
    K_len = n_sites * IF + 1  # +1 = ones column carrying the bias
    K_pad = max(P, math.ceil(K_len / P) * P)
    T = K_pad // P

    # Keff (masked kernel slice) + bias row, zero-padded, split + swizzled.
    km = np.zeros((K_pad, F), np.float32)
    if n_sites:
        kr = kernel.reshape(S, IF, S, F)[:n_sites, :, index, :]
        km[: n_sites * IF] = np.asarray(kr, np.float32).reshape(n_sites * IF, F)
    km[n_sites * IF] = np.asarray(bias[index], np.float32)
    khi, klo = _split_bf16(km)
    ks = np.stack([khi, klo], axis=1)  # (K_pad, 2, F)
    KM2 = np.ascontiguousarray(
        ks.reshape(T, P, 2, F).transpose(1, 0, 2, 3).reshape(P, T * 2 * F)
    )

    inputs = np.asarray(inputs, np.float32)
    cache = np.asarray(cache, np.float32)
    in_maps = []
    for c in range(NCORES):
        rows = slice(c * B_core, (c + 1) * B_core)
        at = np.zeros((K_pad, B_core), np.float32)
        if n_sites:
            at[: n_sites * IF] = (
                cache[rows, :n_sites, :].reshape(B_core, n_sites * IF).T
            )
            at[hi_site * IF : (hi_site + 1) * IF] = inputs[rows].T
        at[n_sites * IF] = 1.0
        ahi, alo = _split_bf16(at)
        at2 = np.empty((T, 2, P, B_core), dtype=ahi.dtype)
        at2[:, 0] = ahi.reshape(T, P, B_core)
        at2[:, 1] = alo.reshape(T, P, B_core)
        in_maps.append({"at2": at2.reshape(T * 2 * P, B_core), "km2": KM2})
    return in_maps, K_pad, B_core, F


def kernel(inputs, cache, kernel, bias, index):
    from concourse.bass_utils import run_bass_kernel_spmd

    in_maps, K_pad, B_core, F = _prepare(inputs, cache, kernel, bias, index)
    nc = _get_nc(K_pad, B_core, F)
    res = run_bass_kernel_spmd(nc, in_maps, core_ids=list(range(NCORES)))
    out = np.concatenate(
        [np.asarray(res.results[c]["out"]).T for c in range(NCORES)], axis=0
    )
    return np.ascontiguousarray(out, dtype=np.float32)


# revision 4
# speedup vs baseline: 215.3317x; 2.5754x over previous
"""Trainium2 Bass kernel for FastMaskedDense1D.update_site (index=300 regime).

Math (reference semantics, EXCLUSIVE=1):
    cache[:, index-1, :] = inputs                      (scatter)
    cache_i = cache[:, :index+1, :].reshape(B, -1)
    kernel_i = kernel.reshape(S, IF, S, F)[:index+1, :, index, :]
    kernel_i *= (arange(index+1) <= index-1)[:, None, None]   (mask)
    y = cache_i @ kernel_i.reshape(-1, F) + bias[index]

Because the mask zeroes site `index`, only sites 0..index-1 contribute, with
site index-1 replaced by `inputs`. So the whole op is one skinny matmul:
    y = A @ Keff + bias,  A: (B, index*IF), Keff: (index*IF, F)

Strategy: data-parallel over the batch across 8 NeuronCores. The host folds
scatter + mask + kernel-slice + bias (ones-column trick) into a per-core
dense problem laid out contraction-major (A^T, contiguous) so each core runs
a single streaming matmul  out^T = Keff^T @ A^T  at full DMA rate.

The PE's native fp32 matmul runs at 4 cycles/row, which would make the
TensorEngine the bottleneck (~65us/core vs ~55us DMA). Instead the host
splits fp32 into an fp16 hi/lo pair (hi = fp16(x), lo = fp16(x - hi)) and the
device computes 3 full-rate fp16 matmuls accumulated in fp32 PSUM:
    A@K ~= Ahi@Khi + Alo@Khi + Ahi@Klo     (lo@lo dropped, ~2^-24 relative)
Same DRAM traffic as fp32 (2 x fp16), ~1e-7 relative error (fp32-grade; the
data is well inside fp16 range), PE off the critical path (~50us < DMA
~55us).

DRAM layout per core:
  at2 (K_pad*2, 1024) fp16 : k-tile-interleaved rows
                             [t0_hi(128) | t0_lo(128) | t1_hi(128) | ...]
  km2 (128, T*2*F) fp16    : km2[p, (t*2+h)*F + n] = Ksplit[t*128+p, h, n]
  out (F, 1024) f32        : out^T; host transposes back
"""

import math

import numpy as np

BATCH = 8192
SIZE = 512
FEATURES = 16
IN_FEATURES = 16
EXCLUSIVE = 1
NCORES = 8
P = 128
G = 4  # contraction k-tiles per DMA (1 MB transfers with split_b=2)
SPLIT_B = 2  # DMAs per supertile along the batch dim (finer dep granularity)
BUFS = 3

_NC_CACHE: dict = {}


def _build(K_pad: int, B: int, F: int, repeats: int = 1):
    """out(F, B) = sum_t Ksplit[t].T @ Asplit[t] over fp16 hi/lo pairs."""
    import concourse.bacc as bacc
    import concourse.mybir as mybir
    from concourse.tile import TileContext

    F32 = mybir.dt.float32
    FP16 = mybir.dt.float16
    T = K_pad // P
    assert B % 512 == 0
    NBH = B // 512

    nc = bacc.Bacc("TRN2", target_bir_lowering=False, debug=False)
    AT2 = nc.dram_tensor("at2", (K_pad * 2, B), FP16, kind="ExternalInput")
    KM2 = nc.dram_tensor("km2", (P, T * 2 * F), FP16, kind="ExternalInput")
    OUT = nc.dram_tensor("out", (F, B), F32, kind="ExternalOutput")

    # q = t*2 + h  (k-tile t, half h: 0=hi, 1=lo)
    at_view = AT2.ap().rearrange("(q p) b -> p q b", p=P)

    with TileContext(nc) as tc:
        with (
            tc.tile_pool(name="kw", bufs=1) as kwpool,
            tc.tile_pool(name="a", bufs=BUFS) as apool,
            tc.tile_pool(name="o", bufs=2) as opool,
            tc.tile_pool(name="ps", bufs=2 * NBH, space="PSUM") as pspool,
        ):
            kw = kwpool.tile([P, T * 2 * F], FP16)
            nc.sync.dma_start(kw[:], KM2.ap())

            def kwt(t, h):
                o = (t * 2 + h) * F
                return kw[:, o : o + F]

            for _ in range(repeats):
                psums = [
                    pspool.tile([F, 512], F32, tag=f"ps{bh}", name=f"psum{bh}")
                    for bh in range(NBH)
                ]
                t = 0
                while t < T:
                    g = min(G, T - t)
                    a_tile = apool.tile([P, G * 2, B], FP16, tag="a", name="a_tile")
                    bs = B // SPLIT_B
                    for sb in range(SPLIT_B):
                        nc.sync.dma_start(
                            a_tile[:, : g * 2, sb * bs : (sb + 1) * bs],
                            at_view[:, t * 2 : (t + g) * 2, sb * bs : (sb + 1) * bs],
                        )
                    for gi in range(g):
                        tt = t + gi
                        for bh in range(NBH):
                            bsl = slice(bh * 512, (bh + 1) * 512)
                            # (K half, A half): (hi,hi), (lo,hi), (hi,lo)
                            for j, (kh, ah) in enumerate(((0, 0), (1, 0), (0, 1))):
                                nc.tensor.matmul(
                                    psums[bh][:],
                                    kwt(tt, kh),
                                    a_tile[:, gi * 2 + ah, bsl],
                                    start=(tt == 0 and j == 0),
                                    stop=(tt == T - 1 and j == 2),
                                )
                    t += g
                outsb = opool.tile([F, B], F32, tag="out", name="outsb")
                for bh in range(NBH):
                    nc.any.tensor_copy(
                        out=outsb[:, bh * 512 : (bh + 1) * 512], in_=psums[bh][:]
                    )
                nc.sync.dma_start(OUT.ap(), outsb[:])
    nc.compile()
    return nc


def _get_nc(K_pad: int, B: int, F: int, repeats: int = 1):
    key = (K_pad, B, F, repeats)
    if key not in _NC_CACHE:
        _NC_CACHE[key] = _build(K_pad, B, F, repeats)
    return _NC_CACHE[key]


def _split_fp16(x):
    hi = x.astype(np.float16)
    lo = (x - hi.astype(np.float32)).astype(np.float16)
    return hi, lo


def _prepare(inputs, cache, kernel, bias, index):
    """Host-side fold: returns (in_maps, K_pad, B_core, F)."""
    index = int(index)
    B, IF = inputs.shape
    S, F = bias.shape
    assert B % NCORES == 0
    B_core = B // NCORES

    hi_site = index - EXCLUSIVE
    n_sites = hi_site + 1 if hi_site >= 0 else 0
    K_len = n_sites * IF + 1  # +1 = ones column carrying the bias
    K_pad = max(P, math.ceil(K_len / P) * P)
    T = K_pad // P

    # Keff (masked kernel slice) + bias row, zero-padded, split + swizzled.
    km = np.zeros((K_pad, F), np.float32)
    if n_sites:
        kr = kernel.reshape(S, IF, S, F)[:n_sites, :, index, :]
        km[: n_sites * IF] = np.asarray(kr, np.float32).reshape(n_sites * IF, F)
    km[n_sites * IF] = np.asarray(bias[index], np.float32)
    khi, klo = _split_fp16(km)
    ks = np.stack([khi, klo], axis=1)  # (K_pad, 2, F)
    KM2 = np.ascontiguousarray(
        ks.reshape(T, P, 2, F).transpose(1, 0, 2, 3).reshape(P, T * 2 * F)
    )

    inputs = np.asarray(inputs, np.float32)
    cache = np.asarray(cache, np.float32)
    in_maps = []
    for c in range(NCORES):
        rows = slice(c * B_core, (c + 1) * B_core)
        at = np.zeros((K_pad, B_core), np.float32)
        if n_sites:
            at[: n_sites * IF] = (
                cache[rows, :n_sites, :].reshape(B_core, n_sites * IF).T
            )
            at[hi_site * IF : (hi_site + 1) * IF] = inputs[rows].T
        at[n_sites * IF] = 1.0
        ahi, alo = _split_fp16(at)
        at2 = np.empty((T, 2, P, B_core), dtype=ahi.dtype)
        at2[:, 0] = ahi.reshape(T, P, B_core)
        at2[:, 1] = alo.reshape(T, P, B_core)
        in_maps.append({"at2": at2.reshape(T * 2 * P, B_core), "km2": KM2})
    return in_maps, K_pad, B_core, F


def kernel(inputs, cache, kernel, bias, index):
    from concourse.bass_utils import run_bass_kernel_spmd

    in_maps, K_pad, B_core, F = _prepare(inputs, cache, kernel, bias, index)
    nc = _get_nc(K_pad, B_core, F)
    res = run_bass_kernel_spmd(nc, in_maps, core_ids=list(range(NCORES)))
    out = np.concatenate(
        [np.asarray(res.results[c]["out"]).T for c in range(NCORES)], axis=0
    )
    return np.ascontiguousarray(out, dtype=np.float32)
